# revision 12
# baseline (speedup 1.0000x reference)
"""Multi-head attention TRN2 kernel: 8-way head-parallel (tensor parallel).

Contract: kernel(**inputs) takes FULL numpy inputs (q,k,v,attn_mask,rel_bias,
Wq,bq,Wk,bk,Wv,bv,Wo,bo) and returns the FULL [B,S,D] float32 output.

Sharding: heads split across 8 cores (2 heads/core, both batches on every
core).  Wq/Wk/Wv column-sliced per head-pair, Wo row-sliced, rel_bias
head-sliced.  Each core computes a partial output projection; the host sums
the 8 partials and adds bo.

On-chip dataflow per core (all matmuls bf16 operands, fp32 PSUM):
  QhT/KhT d-major [128, S] per batch  (d = 2 heads x 64)
  Vh     k-major with fused bias row + ones columns (for softmax row sums)
  scoresT[k, q] computed directly transposed (zero on-chip transposes)
  exp on ACT (1/sqrt(dh) fused into activation scale)
  bias multiply exp(rel_bias)*mask on DVE (host-precomputed, transposed)
  attnV: out^T[d, q]; extra lhsT ones-column accumulates l[q] = sum_k P
  1/l normalization fused into the PSUM eviction (GPSIMD partition-broadcast)
  y = outT^T @ Wo per q-tile, evicted (DVE/ACT alternating) and DMA'd out

Mask handling is exact and generic: blocks are skipped only when the mask
kills an entire [q-block, k-tile] region; within included blocks the host
bakes mask zeros into exp(bias).
"""

import os
import sys
from contextlib import ExitStack

import numpy as np
import ml_dtypes

for _p in ("/opt/trn_rl_repo", "/root/.axon_site/_ro/trn_rl_repo"):
    if os.path.isdir(_p) and _p not in sys.path:
        sys.path.insert(0, _p)

os.environ.setdefault("JAX_COMPILATION_CACHE_DIR", "/tmp/jax_neff_cache")
os.environ.setdefault("JAX_PERSISTENT_CACHE_MIN_COMPILE_TIME_SECS", "1")
os.environ.setdefault("JAX_PERSISTENT_CACHE_MIN_ENTRY_SIZE_BYTES", "0")

import concourse.bass as bass
import concourse.mybir as mybir
import concourse.tile as tile
from concourse import bacc
from concourse.bass_utils import run_bass_kernel_spmd

BF16 = mybir.dt.bfloat16
F32 = mybir.dt.float32
AF = mybir.ActivationFunctionType

B, S, D, H = 2, 2048, 1024, 16
DH = D // H
NCORES = 8
HPC = H // NCORES        # heads per core = 2
DPC = HPC * DH           # feature dim per core = 128
NKT = S // 128           # 16 k-tiles of 128 tokens
NCC = D // 128           # 8 contraction chunks for projections
NQT = S // 128           # q tiles for output
PIECE = 1024             # scores/exp granule width (2 PSUM banks)
HALF = 1024              # attnV accumulator column half-width

_bf = ml_dtypes.bfloat16


# --------------------------------------------------------------------------
# Plan: block structure derived from the actual attention mask.
# --------------------------------------------------------------------------
class Plan:
    def __init__(self, attn_mask):
        m = np.asarray(attn_mask)
        assert m.shape == (B, 1, S, S)
        self.qlo = np.zeros((B, NKT), np.int64)
        self.live = np.zeros((B, NKT), np.int64)
        for b in range(B):
            mb = m[b, 0]
            for kt in range(NKT):
                blk = mb[:, kt * 128:(kt + 1) * 128]
                rows = np.flatnonzero(blk.reshape(NQT, 128, 128).any(axis=(1, 2)))
                if len(rows) == 0:
                    self.qlo[b, kt] = S
                    self.live[b, kt] = 0
                else:
                    self.qlo[b, kt] = int(rows.min()) * 128
                    self.live[b, kt] = S - self.qlo[b, kt]
        self.shared_bias = bool(np.array_equal(m[0], m[1]))
        # PT / expbt column offsets.  When the mask is batch-independent the
        # two batches share one bias image (off[1] == off[0]).
        self.off = np.zeros((B, NKT), np.int64)
        if self.shared_bias:
            o = 0
            for kt in range(NKT):
                self.off[:, kt] = o
                o += self.live[0, kt]
            self.tot = int(o)
        else:
            o = 0
            for b in range(B):
                for kt in range(NKT):
                    self.off[b, kt] = o
                    o += self.live[b, kt]
            self.tot = int(o)
        # per-batch PT tile width (columns a single batch's PT needs)
        self.pt_tot = int(max(
            sum(int(self.live[b, kt]) for kt in range(NKT)) for b in range(B)))
        self.interleave = self.shared_bias and self.pt_tot <= 20000

    def key(self):
        return (tuple(self.qlo.ravel().tolist()),
                tuple(self.live.ravel().tolist()), self.shared_bias)


# --------------------------------------------------------------------------
# Kernel builder
# --------------------------------------------------------------------------
def build(plan: Plan):
    nc = bacc.Bacc("TRN2", target_bir_lowering=False, debug=False)

    xt = [nc.dram_tensor(n, [NCC, 128, B * S], BF16, kind="ExternalInput").ap()
          for n in ("xq", "xk", "xv")]
    wq = nc.dram_tensor("wq", [NCC, 128, DPC], BF16, kind="ExternalInput").ap()
    wk = nc.dram_tensor("wk", [NCC, 128, DPC], BF16, kind="ExternalInput").ap()
    wv = nc.dram_tensor("wv", [NCC, 128, DPC], BF16, kind="ExternalInput").ap()
    bq = nc.dram_tensor("bq", [DPC, 1], F32, kind="ExternalInput").ap()
    bk = nc.dram_tensor("bk", [DPC, 1], F32, kind="ExternalInput").ap()
    bvr = nc.dram_tensor("bvr", [1, DPC], BF16, kind="ExternalInput").ap()
    wo = nc.dram_tensor("wo", [DPC, D], BF16, kind="ExternalInput").ap()
    ebt = nc.dram_tensor("ebt", [128, HPC * plan.tot], BF16,
                         kind="ExternalInput").ap()
    out = nc.dram_tensor("out", [B, NQT, 128, D], F32,
                         kind="ExternalOutput").ap()
    dbg = None
    if getattr(plan, "debug", False):
        dbg = {
            "qhT": nc.dram_tensor("d_qhT", [128, S], BF16, kind="ExternalOutput").ap(),
            "khT": nc.dram_tensor("d_khT", [128, S], BF16, kind="ExternalOutput").ap(),
            "vh": nc.dram_tensor("d_vh", [128, NKT * 256], BF16, kind="ExternalOutput").ap(),
            "pt": nc.dram_tensor("d_pt", [128, max(plan.pt_tot, 1)], BF16, kind="ExternalOutput").ap(),
            "l": nc.dram_tensor("d_l", [128, S], F32, kind="ExternalOutput").ap(),
            "osb": nc.dram_tensor("d_osb", [128, S], BF16, kind="ExternalOutput").ap(),
        }

    with tile.TileContext(nc) as tc, ExitStack() as ctx:
        _body(ctx, tc, plan, xt, wq, wk, wv, bq, bk, bvr, wo, ebt, out, dbg)
    nc.finalize()
    return nc


def _body(ctx, tc, plan, xt, wq, wk, wv, bq, bk, bvr, wo, ebt, out, dbg=None):
    nc = tc.nc

    xt_pool = ctx.enter_context(tc.tile_pool(name="xt", bufs=10))
    w_pool = ctx.enter_context(tc.tile_pool(name="w", bufs=1))
    qk_pool = ctx.enter_context(tc.tile_pool(name="qk", bufs=2))
    vh_pool = ctx.enter_context(tc.tile_pool(name="vh", bufs=2))
    pt_pool = ctx.enter_context(
        tc.tile_pool(name="pt", bufs=2 if plan.interleave or plan.pt_tot <= 20000 else 1))
    ebt_pool = ctx.enter_context(tc.tile_pool(name="ebt", bufs=4))
    es_pool = ctx.enter_context(tc.tile_pool(name="es", bufs=3))
    osb_pool = ctx.enter_context(tc.tile_pool(name="osb", bufs=2))
    y_pool = ctx.enter_context(tc.tile_pool(name="y", bufs=2))
    l_pool = ctx.enter_context(tc.tile_pool(name="l", bufs=2))
    rl_pool = ctx.enter_context(tc.tile_pool(name="rl", bufs=2))
    sm_pool = ctx.enter_context(tc.tile_pool(name="sm", bufs=3))

    ps_proj = ctx.enter_context(tc.tile_pool(name="psp", bufs=1, space="PSUM"))
    ps_sco = ctx.enter_context(tc.tile_pool(name="pss", bufs=2, space="PSUM"))
    ps_acc = ctx.enter_context(tc.tile_pool(name="psa", bufs=1, space="PSUM"))

    # ---------------- weights ----------------
    wq_sb = w_pool.tile([128, NCC * DPC], BF16, tag="wq")
    wk_sb = w_pool.tile([128, NCC * DPC], BF16, tag="wk")
    wv_sb = w_pool.tile([128, NCC * DPC], BF16, tag="wv")
    for cc in range(NCC):
        nc.sync.dma_start(wq_sb[:, cc * DPC:(cc + 1) * DPC], wq[cc])
        nc.sync.dma_start(wk_sb[:, cc * DPC:(cc + 1) * DPC], wk[cc])
        nc.sync.dma_start(wv_sb[:, cc * DPC:(cc + 1) * DPC], wv[cc])
    wo_sb = w_pool.tile([DPC, D], BF16, tag="wo")
    nc.sync.dma_start(wo_sb[:], wo[:])
    bq_sb = w_pool.tile([DPC, 1], F32, tag="bq")
    bk_sb = w_pool.tile([DPC, 1], F32, tag="bk")
    bvr_sb = w_pool.tile([1, DPC], BF16, tag="bvr")
    ones_sb = w_pool.tile([1, 128], BF16, tag="ones")
    nc.sync.dma_start(bq_sb[:], bq[:])
    nc.sync.dma_start(bk_sb[:], bk[:])
    nc.sync.dma_start(bvr_sb[:], bvr[:])
    nc.vector.memset(ones_sb[:], 1.0)

    qhT, khT, vh, outsb, pt = {}, {}, {}, {}, {}
    for b in range(B):
        outsb[b] = osb_pool.tile([128, S], BF16, tag="osb", name=f"outsb{b}")

    # ---------------- projections ----------------
    def proj_b(b):
        tok0 = b * S
        xg = {}
        for i, name in enumerate(("q", "k", "v")):
            for cc in range(NCC):
                t = xt_pool.tile([128, S], BF16, tag="xt")
                nc.sync.dma_start(t[:], xt[i][cc, :, tok0:tok0 + S])
                xg[(name, cc)] = t

        for name, wsb, bias in (("q", wq_sb, bq_sb), ("k", wk_sb, bk_sb)):
            big = qk_pool.tile([128, S], BF16, tag=f"{name}hT")
            (qhT if name == "q" else khT)[b] = big
            for half in range(S // HALF):
                ps = ps_proj.tile([128, HALF], F32, tag="proj")
                for cc in range(NCC):
                    for j in range(HALF // 512):
                        q0 = half * HALF + j * 512
                        nc.tensor.matmul(
                            ps[:, j * 512:(j + 1) * 512],
                            wsb[:, cc * DPC:(cc + 1) * DPC],
                            xg[(name, cc)][:, q0:q0 + 512],
                            start=(cc == 0), stop=(cc == NCC - 1))
                nc.scalar.activation(
                    big[:, half * HALF:(half + 1) * HALF], ps[:],
                    AF.Identity, bias=bias[:])

        # V (k-major): per k-tile 256 cols:
        #   [0:64]=h0 d, [64]=ones(l at out partition 64), [65:128]=unused,
        #   [128]=ones(l at out partition 0), [129:191]=zeros, [192:256]=h1 d
        vext = vh_pool.tile([128, NKT * 256], BF16, tag="vh")
        vh[b] = vext
        nc.vector.memset(vext[:], 0.0)
        for g in range(2):
            ps = ps_proj.tile([128, HALF], F32, tag="proj")
            # NOTE: matmul start=True clears the has_written bits of the
            # ENTIRE psum bank, so it must be issued exactly once per bank
            # (first touch); later matmuls into the bank use start=False
            # (clean bits -> plain write, set bits -> accumulate).
            for cc in range(NCC):
                for j in range(8):
                    kt = g * 8 + j
                    nc.tensor.matmul(
                        ps[:, j * 128:(j + 1) * 128],
                        xg[("v", cc)][:, kt * 128:(kt + 1) * 128],
                        wv_sb[:, cc * DPC:(cc + 1) * DPC],
                        start=(cc == 0 and j % 4 == 0), stop=False)
            for j in range(8):
                nc.tensor.matmul(
                    ps[:, j * 128:(j + 1) * 128],
                    ones_sb[:, :128], bvr_sb[:],
                    start=False, stop=(j % 4 == 3))
            ps3 = ps[:].rearrange("p (t d) -> p t d", d=128)
            dst3 = vext[:, g * 8 * 256:(g + 1) * 8 * 256].rearrange(
                "p (t d) -> p t d", d=256)
            nc.vector.tensor_copy(dst3[:, :, 0:64], ps3[:, :, 0:64])
            nc.vector.tensor_copy(dst3[:, :, 192:256], ps3[:, :, 64:128])
        for kt in range(NKT):
            nc.vector.memset(vext[:, kt * 256 + 64:kt * 256 + 65], 1.0)
            nc.vector.memset(vext[:, kt * 256 + 128:kt * 256 + 129], 1.0)

    # ---------------- attention ----------------
    def scores_granule(b, h, kt, p0, p, eb):
        """one (kt, piece): scoresT matmul -> exp -> bias-mult into PT."""
        qlo = int(plan.qlo[b, kt])
        off = int(plan.off[b, kt])
        d0 = 64 * h
        ps = ps_sco.tile([128, PIECE], F32, tag="sco")
        for j in range(0, p, 512):
            w = min(512, p - j)
            nc.tensor.matmul(
                ps[:, j:j + w],
                khT[b][d0:d0 + 64, kt * 128:(kt + 1) * 128],
                qhT[b][d0:d0 + 64, qlo + p0 + j:qlo + p0 + j + w],
                start=True, stop=True)
        es = es_pool.tile([128, PIECE], BF16, tag="es")
        nc.scalar.activation(es[:, 0:p], ps[:, 0:p], AF.Exp,
                             scale=float(1.0 / np.sqrt(DH)))
        nc.vector.tensor_mul(pt[b][:, off + p0:off + p0 + p],
                             es[:, 0:p], eb[:, 0:p])

    def attn_v(b, h):
        """attnV + l + normalized eviction for (b, h)."""
        for half in range(S // HALF):
            c0, c1 = half * HALF, (half + 1) * HALF
            acc = ps_acc.tile([128, HALF], F32, tag="acc")
            kts = [kt for kt in range(NKT)
                   if plan.live[b, kt] > 0 and plan.qlo[b, kt] < c1]
            kts.sort(key=lambda kt: (int(plan.qlo[b, kt]), kt))
            for i, kt in enumerate(kts):
                qlo = int(plan.qlo[b, kt])
                off = int(plan.off[b, kt])
                lo = max(c0, qlo)
                for j in range(lo, c1, 512):
                    w = min(512, c1 - j)
                    if h == 0:
                        lhsT = vh[b][:, kt * 256:kt * 256 + 65]
                        dst = acc[0:65, j - c0:j - c0 + w]
                    else:
                        lhsT = vh[b][:, kt * 256 + 128:kt * 256 + 256]
                        dst = acc[:, j - c0:j - c0 + w]
                    nc.tensor.matmul(
                        dst, lhsT,
                        pt[b][:, off + (j - qlo):off + (j - qlo) + w],
                        start=(i == 0), stop=(i == len(kts) - 1))
            # l row -> 1/l -> broadcast; eviction scaled by 1/l
            lrow = 64 if h == 0 else 0
            l_sb = l_pool.tile([65, HALF], F32, tag="lsb")
            nc.scalar.copy(l_sb[lrow:lrow + 1, :], acc[lrow:lrow + 1, :])
            if dbg is not None and b == 0 and h == 0:
                nc.sync.dma_start(dbg["l"][lrow:lrow + 1, c0:c1],
                                  l_sb[lrow:lrow + 1, :])
            l128 = sm_pool.tile([128, HALF // 128], F32, tag="l128")
            nc.sync.dma_start(l128[:], l_sb[lrow:lrow + 1, :])
            rl128 = sm_pool.tile([128, HALF // 128], F32, tag="rl128")
            nc.vector.reciprocal(rl128[:], l128[:])
            rlrow = sm_pool.tile([1, HALF], F32, tag="rlrow")
            nc.sync.dma_start(rlrow[:], rl128[:])
            rl_bc = rl_pool.tile([128, HALF], F32, tag="rlbc")
            nc.gpsimd.partition_broadcast(rl_bc[:], rlrow[:])
            r0 = 0 if h == 0 else 64
            nc.vector.tensor_mul(outsb[b][r0:r0 + 64, c0:c1],
                                 acc[r0:r0 + 64, :], rl_bc[r0:r0 + 64, :])

    def attention_h(h):
        """Both batches, sharing each bias granule load (shared masks)."""
        for b in range(B):
            pt[b] = pt_pool.tile([128, max(plan.pt_tot, 1)], BF16, tag="pt", name=f"pt{b}")
        for kt in range(NKT):
            live = int(plan.live[0, kt])
            if live == 0:
                continue
            off = int(plan.off[0, kt])
            p0 = 0
            while p0 < live:
                p = min(PIECE, live - p0)
                eb = ebt_pool.tile([128, PIECE], BF16, tag="ebt")
                ecol = h * plan.tot + off + p0
                nc.sync.dma_start(eb[:, 0:p], ebt[:, ecol:ecol + p])
                for b in range(B):
                    scores_granule(b, h, kt, p0, p, eb)
                p0 += p
        for b in range(B):
            attn_v(b, h)

    def attention_bh(b, h):
        """Separate path when masks differ across batches."""
        pt[b] = pt_pool.tile([128, max(plan.pt_tot, 1)], BF16, tag="pt", name=f"pt{b}")
        for kt in range(NKT):
            live = int(plan.live[b, kt])
            if live == 0:
                continue
            off = int(plan.off[b, kt])
            p0 = 0
            while p0 < live:
                p = min(PIECE, live - p0)
                eb = ebt_pool.tile([128, PIECE], BF16, tag="ebt")
                ecol = h * plan.tot + off + p0
                nc.sync.dma_start(eb[:, 0:p], ebt[:, ecol:ecol + p])
                scores_granule(b, h, kt, p0, p, eb)
                p0 += p
        attn_v(b, h)

    # ---------------- output projection ----------------
    def yout_b(b):
        for qt in range(NQT):
            ps = ps_sco.tile([128, PIECE], F32, tag="sco")
            for j in range(D // 512):
                nc.tensor.matmul(
                    ps[:, j * 512:(j + 1) * 512],
                    outsb[b][:, qt * 128:(qt + 1) * 128],
                    wo_sb[:, j * 512:(j + 1) * 512],
                    start=True, stop=True)
            y = y_pool.tile([128, D], F32, tag="y")
            if qt % 2 == 0:
                nc.vector.tensor_copy(y[:, 0:512], ps[:, 0:512])
                nc.scalar.copy(y[:, 512:1024], ps[:, 512:1024])
            else:
                nc.scalar.copy(y[:, 0:512], ps[:, 0:512])
                nc.vector.tensor_copy(y[:, 512:1024], ps[:, 512:1024])
            nc.sync.dma_start(out[b, qt], y[:])

    def dump_debug():
        if dbg is None:
            return
        nc.sync.dma_start(dbg["qhT"][:], qhT[0][:])
        nc.sync.dma_start(dbg["khT"][:], khT[0][:])
        nc.sync.dma_start(dbg["vh"][:], vh[0][:])
        nc.sync.dma_start(dbg["pt"][:], pt[0][:])
        nc.sync.dma_start(dbg["osb"][:], outsb[0][:])

    # ---------------- schedule ----------------
    for _rep in range(getattr(plan, "reps", 1)):
        if plan.interleave:
            proj_b(0)
            proj_b(1)
            attention_h(0)
            dump_debug()
            attention_h(1)
            yout_b(0)
            yout_b(1)
        else:
            proj_b(0)
            attention_bh(0, 0)
            attention_bh(0, 1)
            proj_b(1)
            yout_b(0)
            attention_bh(1, 0)
            attention_bh(1, 1)
            yout_b(1)


# --------------------------------------------------------------------------
# Host side
# --------------------------------------------------------------------------
_CACHE = {}


def _prep(plan, q, k, v, rel_bias, attn_mask, Wq, bq_, Wk, bk_, Wv, bv_, Wo):
    xqT = np.ascontiguousarray(
        q.reshape(B * S, D).T.astype(_bf)).reshape(NCC, 128, B * S)
    xkT = np.ascontiguousarray(
        k.reshape(B * S, D).T.astype(_bf)).reshape(NCC, 128, B * S)
    xvT = np.ascontiguousarray(
        v.reshape(B * S, D).T.astype(_bf)).reshape(NCC, 128, B * S)
    mask = np.asarray(attn_mask)
    in_maps = []
    for c in range(NCORES):
        d0 = c * DPC
        ebt = np.zeros((128, HPC * plan.tot), _bf)
        for jh in range(HPC):
            h = c * HPC + jh
            eb = np.exp(rel_bias[h].astype(np.float64)).astype(np.float32)
            bs = [0] if plan.shared_bias else range(B)
            for b in bs:
                ebm = np.where(mask[b, 0], eb, 0.0).astype(np.float32)
                for kt in range(NKT):
                    live = int(plan.live[b, kt])
                    if live == 0:
                        continue
                    qlo = int(plan.qlo[b, kt])
                    off = jh * plan.tot + int(plan.off[b, kt])
                    blkT = ebm[qlo:, kt * 128:(kt + 1) * 128].T
                    ebt[:, off:off + live] = blkT.astype(_bf)
        m = {
            "xq": xqT, "xk": xkT, "xv": xvT,
            "wq": np.ascontiguousarray(
                Wq[:, d0:d0 + DPC].astype(_bf)).reshape(NCC, 128, DPC),
            "wk": np.ascontiguousarray(
                Wk[:, d0:d0 + DPC].astype(_bf)).reshape(NCC, 128, DPC),
            "wv": np.ascontiguousarray(
                Wv[:, d0:d0 + DPC].astype(_bf)).reshape(NCC, 128, DPC),
            "bq": bq_[d0:d0 + DPC].astype(np.float32).reshape(DPC, 1),
            "bk": bk_[d0:d0 + DPC].astype(np.float32).reshape(DPC, 1),
            "bvr": bv_[d0:d0 + DPC].astype(_bf).reshape(1, DPC),
            "wo": np.ascontiguousarray(Wo[d0:d0 + DPC].astype(_bf)),
            "ebt": ebt,
        }
        in_maps.append(m)
    return in_maps


def kernel(q, k, v, attn_mask, rel_bias, Wq, bq, Wk, bk, Wv, bv, Wo, bo):
    q = np.asarray(q, np.float32)
    k = np.asarray(k, np.float32)
    v = np.asarray(v, np.float32)
    rel_bias = np.asarray(rel_bias, np.float32)
    plan = Plan(attn_mask)
    key = plan.key()
    if key not in _CACHE:
        _CACHE[key] = build(plan)
    nc = _CACHE[key]
    in_maps = _prep(plan, q, k, v, rel_bias, attn_mask,
                    np.asarray(Wq, np.float32), np.asarray(bq, np.float32),
                    np.asarray(Wk, np.float32), np.asarray(bk, np.float32),
                    np.asarray(Wv, np.float32), np.asarray(bv, np.float32),
                    np.asarray(Wo, np.float32))
    res = run_bass_kernel_spmd(nc, in_maps, list(range(NCORES)))
    y = np.zeros((B, NQT, 128, D), np.float64)
    for r in res.results:
        y += r["out"].astype(np.float64)
    y = y.reshape(B, S, D) + np.asarray(bo, np.float64)[None, None, :]
    return y.astype(np.float32)
